# revision 39
# baseline (speedup 1.0000x reference)
"""Causal multi-head attention on 8 Trainium2 NeuronCores.

Problem: B=2, S=2048, D=1024, H=16, Dh=64 (fp32 in/out).
Sharding: core c handles batch b = c//4 and 4 heads [4g, 4g+4), g = c%4
(data parallel over batch x head-group tensor parallel). Each core returns
a partial attention output (its heads' z @ W_O); the host sums the 4 partials
per batch and adds the bias terms.

All matmul operands are bf16 (hosts converts inputs; PSUM accumulates fp32):
bf16 streams 1 row/cycle at the warm 2.4 GHz PE clock and enables FWL,
whereas fp32 runs the 2-pass HIGH/LOW path. End-to-end numeric check
(CPU-simulated rounding) gives rel err ~3.5e-3 against the fp32 reference.

On-core layout (everything transposed so no on-chip transposes are needed):
  x^T [d, s] comes pre-transposed from the host (bf16).
  Q^T, K^T [e, s] per head-pair (head A partitions 0-63, head B 64-127),
  produced by matmul(lhsT=W[d, e2], rhs=x^T[d, s]), bias added on DVE with
  bf16 output.
  V_aug [s, 128] per head: cols 0-63 = V (bf16), cols 64-127 = 1.0, so the
  z-matmul deposits the softmax denominator REPLICATED on PSUM partitions
  64-127 (same cycle count - matmul time is rows-streamed only). The
  normalization is then a full-width DVE reciprocal + multiply; no
  1-partition copies and no gpsimd partition_broadcast anywhere.
  scores^T [k, q] = matmul(lhsT=K^T[e,k-tile], rhs=Q^T[e,q-tile]); the K=64
  contraction packs the two heads of a pair into disjoint PE row groups.
  Diagonal blocks only compute columns >= the causal boundary.
  E = exp(scores^T) with no max subtraction (logits are O(3), exp is safe);
  causal masking multiplies the diagonal 128-blocks by a 0/1 mask slice.
  z_aug [128, q] = matmul(lhsT=V_aug[k, 128], rhs=E[k, q]) accumulated over
  k-tiles; rows 64-127 are the softmax denominator (replicated).
  out [s, d] = matmul(lhsT=z^T[e2, s-tile], rhs=W_O[e2, d]) accumulated over
  the two head pairs.

Scale 1/sqrt(Dh) is folded into W_Q/b_Q on the host. b_V's contribution
(sum_h b_V[h] @ W_O[h], constant per row since softmax weights sum to 1)
and b_O are added on the host.
"""

import numpy as np

B, S, D, H, Dh = 2, 2048, 1024, 16, 64
NCORES = 8
CORES_PER_BATCH = 4
HPC = 4          # heads per core (= 2 pairs)
NPAIR = 2
DT_TILES = 8     # 1024 / 128
ST128 = 16       # s tiles of 128
SQ = 512         # q tile width
NQ4 = 4          # q tiles of 512

_BUILT = None


def _build():
    import concourse.bacc as bacc
    import concourse.mybir as mybir
    import concourse.tile as tile

    f32 = mybir.dt.float32
    bf16 = mybir.dt.bfloat16
    EXP = mybir.ActivationFunctionType.Exp
    COPY = mybir.ActivationFunctionType.Copy

    nc = bacc.Bacc(None)

    xT = nc.dram_tensor("xT", [DT_TILES, 128, S], bf16, kind="ExternalInput")
    # wqk[p] = [wq_p | wk_p] along columns; wo packs both pairs; bqk packs
    # the four bias columns - fewer DMA issues (issue time ~0.7us each paces
    # the startup)
    wqk = nc.dram_tensor("wqk", [NPAIR, 128, 2048], bf16, kind="ExternalInput")
    wv = nc.dram_tensor("wv", [128, 2048], bf16, kind="ExternalInput")
    wo = nc.dram_tensor("wo", [128, 2048], bf16, kind="ExternalInput")
    bqk = nc.dram_tensor("bqk", [128, 4], f32, kind="ExternalInput")
    maskd = nc.dram_tensor("maskd", [128, 896], bf16, kind="ExternalInput")
    out = nc.dram_tensor("out", [ST128, 128, 1024], f32, kind="ExternalOutput")

    with tile.TileContext(nc) as tc:
        with (
            tc.tile_pool(name="const", bufs=1) as constp,
            tc.tile_pool(name="qkst", bufs=1) as qkstp,
            tc.tile_pool(name="xchunk", bufs=8) as xp,
            tc.tile_pool(name="work", bufs=3) as workp,
            tc.tile_pool(name="ps", bufs=1, space="PSUM") as ps,
        ):
            # ---- persistent constants / weights ----
            wqk_sb = [constp.tile([128, 2048], bf16, tag=f"wqk{p}",
                                  name=f"wqk{p}") for p in range(NPAIR)]
            wq_sb = [wqk_sb[p][:, 0:1024] for p in range(NPAIR)]
            wk_sb = [wqk_sb[p][:, 1024:2048] for p in range(NPAIR)]
            wv_sb = constp.tile([128, 2048], bf16, tag="wv", name="wv_sb")
            wo_sb_ = constp.tile([128, 2048], bf16, tag="wo", name="wo_sb")
            wo_sb = [wo_sb_[:, 1024 * p:1024 * p + 1024]
                     for p in range(NPAIR)]
            bqk_sb = constp.tile([128, 4], f32, tag="bqk", name="bqk_sb")
            bq_sb = [bqk_sb[:, 2 * p:2 * p + 1] for p in range(NPAIR)]
            bk_sb = [bqk_sb[:, 2 * p + 1:2 * p + 2] for p in range(NPAIR)]
            mask_sb = constp.tile([128, 896], bf16, tag="mask", name="mask_sb")

            # gpsimd's DMA queue is a SLOW software DGE - nothing
            # startup-critical goes there. Granular transfers (128-256KB)
            # across the two fast queues keep the first matmuls fed early;
            # wq0 goes out first, split in halves.
            nc.scalar.dma_start(wq_sb[0][:, 0:512], wqk[0][:, 0:512])

            xc0 = []
            for t in range(DT_TILES):
                c0 = xp.tile([128, SQ], bf16, tag="x", name=f"x0_{t}")
                if t < 6:
                    nc.sync.dma_start(c0[:], xT[t][:, 0:SQ])
                xc0.append(c0)
            nc.scalar.dma_start(xc0[6][:], xT[6][:, 0:SQ])
            nc.scalar.dma_start(xc0[7][:], xT[7][:, 0:SQ])

            nc.scalar.dma_start(wq_sb[0][:, 512:1024], wqk[0][:, 512:1024])
            nc.scalar.dma_start(wk_sb[0][:, 0:512], wqk[0][:, 1024:1536])
            nc.scalar.dma_start(wk_sb[0][:, 512:1024], wqk[0][:, 1536:2048])
            nc.scalar.dma_start(wv_sb[:, 0:1024], wv[:, 0:1024])
            nc.scalar.dma_start(wv_sb[:, 1024:2048], wv[:, 1024:2048])
            nc.scalar.dma_start(wq_sb[1], wqk[1][:, 0:1024])
            nc.scalar.dma_start(wk_sb[1], wqk[1][:, 1024:2048])
            nc.scalar.dma_start(bqk_sb[:], bqk[:])

            # x for blocks 1-3: one wide DMA per d-tile; first tiles on the
            # fast sync queue
            nc.gpsimd.dma_start(mask_sb[:], maskd[:])
            xf = [qkstp.tile([128, S - SQ], bf16, tag=f"xf{t}", name=f"xf{t}")
                  for t in range(DT_TILES)]
            for t in range(DT_TILES):
                (nc.sync if t < 5 else nc.gpsimd).dma_start(
                    xf[t][:], xT[t][:, SQ:S])

            # PE warm-up: the HAM clock gate starts at 1.2 GHz and needs
            # ~3.4us of sustained activity to reach 2.4 GHz. Burn dummy
            # matmuls on a memset scratch while the first DMAs land so the
            # real projections start at full clock.
            # 15 dummies span the whole input-DMA window (~6.5us cold) so
            # the PE never idles (and re-cools) while transfers land
            warm = workp.tile([128, 512], bf16, tag="warm", bufs=1,
                              name="warm")
            nc.vector.memset(warm[:], 0.0)
            wps = ps.tile([128, 512], f32, tag="v", bufs=2, name="warmps")
            for i in range(15):
                nc.tensor.matmul(wps[:], warm[:, 0:128], warm[:],
                                 start=True, stop=True,
                                 skip_group_check=True)

            # ---- persistent activations ----
            qt_sb = [qkstp.tile([128, S], bf16, tag=f"qt{p}", name=f"qt{p}")
                     for p in range(NPAIR)]
            kt_sb = [qkstp.tile([128, S], bf16, tag=f"kt{p}", name=f"kt{p}")
                     for p in range(NPAIR)]
            zt_sb = [qkstp.tile([128, S], bf16, tag=f"zt{p}", name=f"zt{p}")
                     for p in range(NPAIR)]
            # V, augmented: head h at cols [128h, 128h+64), ones at
            # [128h+64, 128h+128) so the z-matmul replicates the softmax
            # denominator across PSUM partitions 64-127
            v_sb = [qkstp.tile([128, 512], bf16, tag=f"v{kt}", name=f"v{kt}")
                    for kt in range(ST128)]
            for kt in range(ST128):
                vo = v_sb[kt].rearrange("p (h c) -> p h c", c=128)
                ones_src = mask_sb[:, 512:768].rearrange(
                    "p (h c) -> p h c", c=64)
                nc.vector.tensor_copy(vo[:, :, 64:128], ones_src)

            # ---- phase 1 (per 512-wide s block): projections ----
            # returns a list of thunks so the caller can weave projection
            # chunks between attention steps (fills PE bubbles while exp runs)
            def proj_chunks(s4, xc_pre=None):
                sl = slice(s4 * SQ, (s4 + 1) * SQ)
                if xc_pre is not None:
                    xc = xc_pre
                else:
                    fsl = slice(s4 * SQ - SQ, (s4 + 1) * SQ - SQ)
                    xc = [xf[t][:, fsl] for t in range(DT_TILES)]

                def qk_chunk(p):
                    qk_ps = ps.tile([128, 2 * SQ], f32, tag="s", bufs=2,
                                    name=f"qkps{s4}{p}")
                    for t in range(DT_TILES):
                        nc.tensor.matmul(qk_ps[:, 0:SQ], wq_sb[p][:, 128 * t:128 * t + 128], xc[t][:],
                                         start=(t == 0), stop=(t == DT_TILES - 1))
                    for t in range(DT_TILES):
                        nc.tensor.matmul(qk_ps[:, SQ:2 * SQ], wk_sb[p][:, 128 * t:128 * t + 128], xc[t][:],
                                         start=(t == 0), stop=(t == DT_TILES - 1))
                    nc.vector.tensor_scalar_add(qt_sb[p][:, sl], qk_ps[:, 0:SQ],
                                                bq_sb[p][:, 0:1])
                    nc.vector.tensor_scalar_add(kt_sb[p][:, sl], qk_ps[:, SQ:2 * SQ],
                                                bk_sb[p][:, 0:1])

                def v_chunk(j):
                    kt = 4 * s4 + j
                    v_ps = ps.tile([128, 256], f32, tag="v", bufs=2, name=f"vps{kt}")
                    for t in range(DT_TILES):
                        nc.tensor.matmul(v_ps[:],
                                         xc[t][:, j * 128:(j + 1) * 128],
                                         wv_sb[:, 256 * t:256 * t + 256],
                                         start=(t == 0), stop=(t == DT_TILES - 1))
                    vo = v_sb[kt].rearrange("p (h c) -> p h c", c=128)
                    vs = v_ps.rearrange("p (h c) -> p h c", c=64)
                    nc.vector.tensor_copy(vo[:, :, 0:64], vs[:])

                return ([("qk", lambda p=p: qk_chunk(p)) for p in range(NPAIR)]
                        + [("v", lambda j=j: v_chunk(j)) for j in range(4)])

            # ---- phase 3 (emitted interleaved): output projection ----
            def oproj_group(st, half, otag, scalar_copy=False):
                ssl = slice(st * 128, (st + 1) * 128)
                dsl = slice(half * 512, (half + 1) * 512)
                o_ps = ps.tile([128, 512], f32, tag=otag,
                               bufs=2, name=f"ops{st}{half}")
                for p in range(NPAIR):
                    nc.tensor.matmul(o_ps[:], zt_sb[p][:, ssl],
                                     wo_sb[p][:, dsl],
                                     start=(p == 0), stop=(p == NPAIR - 1))
                o_sb = workp.tile([128, 512], f32, tag="osb",
                                  name=f"osb{st}{half}")
                if scalar_copy:
                    nc.scalar.activation(o_sb[:], o_ps[:], COPY)
                else:
                    nc.vector.tensor_copy(o_sb[:], o_ps[:])
                nc.sync.dma_start(out[st][:, dsl], o_sb[:])

            def emit_oproj(q4, otag="z", alt_copy=False):
                # alt_copy spreads the PSUM->SBUF copies over vector+scalar
                # (only safe when no exps remain on the scalar queue)
                i = 0
                for st in range(4 * q4, 4 * q4 + 4):
                    for half in range(2):
                        oproj_group(st, half, otag, alt_copy and i % 2 == 0)
                        i += 1

            def oproj_chunks(q4, otag):
                return [("v", lambda st=st, half=half: oproj_group(
                            st, half, otag))
                        for st in range(4 * q4, 4 * q4 + 4)
                        for half in range(2)]

            # ---- phase 2 (per 512-wide q block): attention ----
            def emit_attn(q4, weave):
                q0 = q4 * SQ
                qsl = slice(q0, q0 + SQ)
                nk = q4 * 4 + 4
                # qk chunks go at the head boundary (the block-end leftovers
                # of the previous weave already covered the previous norms);
                # then the previous block's output projection (tag z, whose
                # buffers those norms freed). The v chunks (own tag) weave
                # into the stream as PE filler, RESERVING two chunks for the
                # block end so the next block's head never waits on a bare
                # norm chain.
                while len(weave) > 2 and weave[0][0] != "v":
                    weave.pop(0)[1]()
                if 0 < q4 < NQ4 - 1:
                    emit_oproj(q4 - 1)
                weave = [c for t, c in weave]
                steps = NPAIR * nk
                per = max(1, -(-steps // max(1, len(weave) - 2)))
                step = 0
                for p in range(NPAIR):
                    za = ps.tile([128, SQ], f32, tag="z", bufs=2, name=f"za{q4}{p}")
                    zb = ps.tile([128, SQ], f32, tag="z", bufs=2, name=f"zb{q4}{p}")
                    for kt in range(nk):
                        ksl = slice(kt * 128, (kt + 1) * 128)
                        d = kt * 128 - q0
                        first = (kt == 0)
                        s_ps = ps.tile([128, 2 * SQ], f32, tag="s",
                                       bufs=2, name=f"sps{q4}{p}{kt}")
                        e = workp.tile([128, 2 * SQ], bf16, tag="e",
                                       bufs=6, name=f"e{q4}{p}{kt}")
                        if d < 0:  # fully-allowed block: full-width scores
                            nc.tensor.matmul(s_ps[:, 0:SQ],
                                             kt_sb[p][0:64, ksl],
                                             qt_sb[p][0:64, qsl])
                            nc.tensor.matmul(s_ps[:, SQ:2 * SQ],
                                             kt_sb[p][64:128, ksl],
                                             qt_sb[p][64:128, qsl])
                            nc.scalar.activation(e[:], s_ps[:], EXP)
                        else:  # diagonal: only columns >= d survive the mask
                            nc.tensor.matmul(s_ps[:, d:SQ],
                                             kt_sb[p][0:64, ksl],
                                             qt_sb[p][0:64, q0 + d:q0 + SQ])
                            nc.tensor.matmul(s_ps[:, SQ + d:2 * SQ],
                                             kt_sb[p][64:128, ksl],
                                             qt_sb[p][64:128, q0 + d:q0 + SQ])
                            # one exp over both heads' trimmed ranges (3D AP)
                            e3 = e.rearrange("p (s q) -> p s q", q=SQ)
                            s3 = s_ps.rearrange("p (s q) -> p s q", q=SQ)
                            nc.scalar.activation(e3[:, :, d:SQ],
                                                 s3[:, :, d:SQ], EXP)
                        for sub, zps in ((0, za), (1, zb)):
                            h = 2 * p + sub
                            vap = v_sb[kt][:, 128 * h:128 * h + 128]
                            ebase = sub * SQ
                            if d < 0:  # fully-allowed block
                                nc.tensor.matmul(
                                    zps[:], vap, e[:, ebase:ebase + SQ],
                                    start=first, stop=False,
                                    skip_group_check=True)
                            else:
                                # columns [0, d) fully masked: skip.
                                # columns [d, d+128): mixed - mask-multiply.
                                em = workp.tile([128, 128], bf16, tag="em",
                                                bufs=6, name=f"em{q4}{p}{kt}{sub}")
                                nc.vector.tensor_mul(
                                    em[:], e[:, ebase + d:ebase + d + 128],
                                    mask_sb[:, 384:512])
                                nc.tensor.matmul(
                                    zps[:, d:d + 128], vap, em[:],
                                    start=first, stop=True,
                                    skip_group_check=True)
                                # columns [d+128, 512): fully allowed.
                                if d + 128 < SQ:
                                    nc.tensor.matmul(
                                        zps[:, d + 128:SQ], vap,
                                        e[:, ebase + d + 128:ebase + SQ],
                                        start=first, stop=False,
                                        skip_group_check=True)
                        step += 1
                        if len(weave) > 2 and step % per == 0:
                            weave.pop(0)()
                    if q4 == NQ4 - 1 and p == NPAIR - 1:
                        # the very last normalization gates the final output
                        # projection: chunk it per 128-col s-tile with the
                        # copies on the (now exp-free) scalar engine, so the
                        # oproj that follows pipelines with it per s-tile
                        for j in range(4):
                            csl = slice(j * 128, (j + 1) * 128)
                            gsl = slice(q0 + j * 128, q0 + (j + 1) * 128)
                            for sub, zps in ((0, za), (1, zb)):
                                den = workp.tile([64, 128], f32, tag="den",
                                                 bufs=4, name=f"dc{j}{sub}")
                                nc.scalar.activation(den[:],
                                                     zps[64:128, csl], COPY)
                                rb = workp.tile([64, 128], f32, tag="rb",
                                                bufs=4, name=f"rc{j}{sub}")
                                nc.vector.reciprocal_approx_fast(rb[:], den[:])
                                nc.vector.tensor_mul(
                                    zt_sb[p][64 * sub:64 * sub + 64, gsl],
                                    zps[0:64, csl], rb[:])
                    else:
                        for sub, zps in ((0, za), (1, zb)):
                            # rows 64-127 hold the denominator replicated by
                            # the ones half of V_aug: DVE copy to SBUF
                            # (custom-DVE recip can't read PSUM; keep scalar
                            # free for exp), DVE recip, full-width multiply
                            den = workp.tile([64, SQ], f32, tag="den", bufs=4,
                                             name=f"den{q4}{p}{sub}")
                            nc.vector.tensor_copy(den[:], zps[64:128, :])
                            rb = workp.tile([64, SQ], f32, tag="rb", bufs=4,
                                            name=f"rb{q4}{p}{sub}")
                            nc.vector.reciprocal_approx_fast(rb[:], den[:])
                            nc.vector.tensor_mul(
                                zt_sb[p][64 * sub:64 * sub + 64, qsl],
                                zps[0:64, :], rb[:])
                    # cover the pair-boundary PSUM recycling stall with a
                    # woven chunk (pure PE work with its own PSUM tag)
                    if p == 0 and len(weave) > 2:
                        weave.pop(0)()
                # block-end leftovers: PE filler between this block's last z
                # matmuls / norms and whatever the caller emits next
                while weave:
                    weave.pop(0)()

            # fully interleave: projections for x-block s4+1 are woven into
            # the attention stream of q-block s4 (whose deps only reach s4),
            # so the PSUM tag rotation flows without a phase barrier and
            # matmul-dense projection chunks fill exp-bound PE bubbles.
            for _, c in proj_chunks(0, xc_pre=xc0):
                c()
            # W_O not needed until the first o-proj
            nc.gpsimd.dma_start(wo_sb_[:], wo[:])
            for q4 in range(NQ4):
                if q4 + 1 < NQ4:
                    nxt = proj_chunks(q4 + 1)
                else:
                    # the last block has no projections left to weave; use
                    # the previous block's output projection (the idle "v"
                    # PSUM rotation) as its PE filler instead
                    nxt = oproj_chunks(q4 - 1, otag="v")
                emit_attn(q4, nxt)
            emit_oproj(NQ4 - 1, otag="v", alt_copy=True)

    nc.compile()
    return nc


def _get_built():
    global _BUILT
    if _BUILT is None:
        _BUILT = _build()
    return _BUILT


def _host_prep(x, W_Q, W_K, W_V, W_O, b_Q, b_K):
    """Build the 8 per-core input maps."""
    import ml_dtypes
    bf16 = ml_dtypes.bfloat16
    scale = np.float32(1.0 / np.sqrt(Dh))
    mask = (np.arange(896)[None, :] >= (np.arange(128)[:, None] + 384)
            ).astype(bf16)
    in_maps = []
    for c in range(NCORES):
        b = c // CORES_PER_BATCH
        g = c % CORES_PER_BATCH
        hs = slice(HPC * g, HPC * g + HPC)
        xT_b = np.ascontiguousarray(x[b].T).reshape(
            DT_TILES, 128, S).astype(bf16)
        def pack_de(w):
            # [4 heads, D, Dh] -> pair-stacked [2, D, 128] -> [2, 128, 8*128]
            a = w.reshape(NPAIR, 2, D, Dh).transpose(0, 2, 1, 3).reshape(
                NPAIR, DT_TILES, 128, 128)
            return np.ascontiguousarray(a.transpose(0, 2, 1, 3)).reshape(
                NPAIR, 128, 1024).astype(bf16)

        wq_c = pack_de(W_Q[hs] * scale)
        wk_c = pack_de(W_K[hs])
        wqk_c = np.ascontiguousarray(
            np.concatenate([wq_c, wk_c], axis=2))
        wv_c = np.ascontiguousarray(
            W_V[hs].transpose(1, 0, 2).reshape(DT_TILES, 128, HPC * Dh)
            .transpose(1, 0, 2)).reshape(128, 2048).astype(bf16)
        wo_p = np.ascontiguousarray(W_O[hs]).reshape(NPAIR, 128, 1024)
        wo_c = np.ascontiguousarray(
            np.concatenate([wo_p[0], wo_p[1]], axis=1)).astype(bf16)
        bq_c = np.ascontiguousarray(b_Q[hs] * scale).reshape(NPAIR, 128)
        bk_c = np.ascontiguousarray(b_K[hs]).reshape(NPAIR, 128)
        bqk_c = np.ascontiguousarray(np.stack(
            [bq_c[0], bk_c[0], bq_c[1], bk_c[1]], axis=1)).astype(np.float32)
        in_maps.append({
            "xT": xT_b, "wqk": wqk_c, "wv": wv_c, "wo": wo_c,
            "bqk": bqk_c, "maskd": mask,
        })
    return in_maps


def kernel(normalized_resid_pre, W_Q, W_K, W_V, W_O, b_Q, b_K, b_V, b_O,
           _want_profile=False):
    from concourse.bass_utils import run_bass_kernel_spmd

    x = np.asarray(normalized_resid_pre, np.float32)
    W_Q = np.asarray(W_Q, np.float32)
    W_K = np.asarray(W_K, np.float32)
    W_V = np.asarray(W_V, np.float32)
    W_O = np.asarray(W_O, np.float32)
    b_Q = np.asarray(b_Q, np.float32)
    b_K = np.asarray(b_K, np.float32)
    b_V = np.asarray(b_V, np.float32)
    b_O = np.asarray(b_O, np.float32)

    in_maps = _host_prep(x, W_Q, W_K, W_V, W_O, b_Q, b_K)
    nc = _get_built()
    kw = {}
    if _want_profile:
        kw = dict(trace=True)
    res = run_bass_kernel_spmd(nc, in_maps, list(range(NCORES)), **kw)

    # host-side unshard: sum the head-group partials per batch + bias terms
    b_eff = b_O + np.einsum("he,hed->d", b_V, W_O).astype(np.float32)
    attn_out = np.zeros((B, S, D), np.float32)
    for c in range(NCORES):
        b = c // CORES_PER_BATCH
        attn_out[b] += res.results[c]["out"].reshape(S, D)
    attn_out += b_eff[None, None, :]
    if _want_profile:
        return attn_out, res
    return attn_out


# revision 40
# speedup vs baseline: 1.1963x; 1.1963x over previous
"""Causal multi-head attention on 8 Trainium2 NeuronCores.

Problem: B=2, S=2048, D=1024, H=16, Dh=64 (fp32 in/out).
Sharding: core c handles batch b = c//4 and 4 heads [4g, 4g+4), g = c%4
(data parallel over batch x head-group tensor parallel). Each core returns
a partial attention output (its heads' z @ W_O); the host sums the 4 partials
per batch and adds the bias terms.

All matmul operands are bf16 (hosts converts inputs; PSUM accumulates fp32):
bf16 streams 1 row/cycle at the warm 2.4 GHz PE clock and enables FWL,
whereas fp32 runs the 2-pass HIGH/LOW path. End-to-end numeric check
(CPU-simulated rounding) gives rel err ~3.5e-3 against the fp32 reference.

On-core layout (everything transposed so no on-chip transposes are needed):
  x^T [d, s] comes pre-transposed from the host (bf16).
  Q^T, K^T [e, s] per head-pair (head A partitions 0-63, head B 64-127),
  produced by matmul(lhsT=W[d, e2], rhs=x^T[d, s]), bias added on DVE with
  bf16 output.
  V_aug [s, 128] per head: cols 0-63 = V (bf16), cols 64-127 = 1.0, so the
  z-matmul deposits the softmax denominator REPLICATED on PSUM partitions
  64-127 (same cycle count - matmul time is rows-streamed only). The
  normalization is then a full-width DVE reciprocal + multiply; no
  1-partition copies and no gpsimd partition_broadcast anywhere.
  scores^T [k, q] = matmul(lhsT=K^T[e,k-tile], rhs=Q^T[e,q-tile]); the K=64
  contraction packs the two heads of a pair into disjoint PE row groups.
  Diagonal blocks only compute columns >= the causal boundary.
  E = exp(scores^T) with no max subtraction (logits are O(3), exp is safe);
  causal masking multiplies the diagonal 128-blocks by a 0/1 mask slice.
  z_aug [128, q] = matmul(lhsT=V_aug[k, 128], rhs=E[k, q]) accumulated over
  k-tiles; rows 64-127 are the softmax denominator (replicated).
  out [s, d] = matmul(lhsT=z^T[e2, s-tile], rhs=W_O[e2, d]) accumulated over
  the two head pairs.

Scale 1/sqrt(Dh) is folded into W_Q/b_Q on the host. b_V's contribution
(sum_h b_V[h] @ W_O[h], constant per row since softmax weights sum to 1)
and b_O are added on the host.
"""

import numpy as np

B, S, D, H, Dh = 2, 2048, 1024, 16, 64
NCORES = 8
CORES_PER_BATCH = 4
HPC = 4          # heads per core (= 2 pairs)
NPAIR = 2
DT_TILES = 8     # 1024 / 128
ST128 = 16       # s tiles of 128
SQ = 512         # q tile width
NQ4 = 4          # q tiles of 512

_BUILT = None


def _build():
    import concourse.bacc as bacc
    import concourse.mybir as mybir
    import concourse.tile as tile

    f32 = mybir.dt.float32
    bf16 = mybir.dt.bfloat16
    EXP = mybir.ActivationFunctionType.Exp
    COPY = mybir.ActivationFunctionType.Copy

    nc = bacc.Bacc(None)

    xT = nc.dram_tensor("xT", [DT_TILES, 128, S], bf16, kind="ExternalInput")
    # wqk[p] = [wq_p | wk_p] along columns; wo packs both pairs; bqk packs
    # the four bias columns - fewer DMA issues (issue time ~0.7us each paces
    # the startup)
    wqk = nc.dram_tensor("wqk", [NPAIR, 128, 2048], bf16, kind="ExternalInput")
    wv = nc.dram_tensor("wv", [128, 2048], bf16, kind="ExternalInput")
    wo = nc.dram_tensor("wo", [128, 2048], bf16, kind="ExternalInput")
    bqk = nc.dram_tensor("bqk", [128, 4], f32, kind="ExternalInput")
    maskd = nc.dram_tensor("maskd", [128, 896], bf16, kind="ExternalInput")
    out = nc.dram_tensor("out", [ST128, 128, 1024], f32, kind="ExternalOutput")

    with tile.TileContext(nc) as tc:
        with (
            tc.tile_pool(name="const", bufs=1) as constp,
            tc.tile_pool(name="qkst", bufs=1) as qkstp,
            tc.tile_pool(name="xchunk", bufs=8) as xp,
            tc.tile_pool(name="work", bufs=3) as workp,
            tc.tile_pool(name="ps", bufs=1, space="PSUM") as ps,
        ):
            # ---- persistent constants / weights ----
            wqk_sb = [constp.tile([128, 2048], bf16, tag=f"wqk{p}",
                                  name=f"wqk{p}") for p in range(NPAIR)]
            wq_sb = [wqk_sb[p][:, 0:1024] for p in range(NPAIR)]
            wk_sb = [wqk_sb[p][:, 1024:2048] for p in range(NPAIR)]
            wv_sb = constp.tile([128, 2048], bf16, tag="wv", name="wv_sb")
            wo_sb_ = constp.tile([128, 2048], bf16, tag="wo", name="wo_sb")
            wo_sb = [wo_sb_[:, 1024 * p:1024 * p + 1024]
                     for p in range(NPAIR)]
            bqk_sb = constp.tile([128, 4], f32, tag="bqk", name="bqk_sb")
            bq_sb = [bqk_sb[:, 2 * p:2 * p + 1] for p in range(NPAIR)]
            bk_sb = [bqk_sb[:, 2 * p + 1:2 * p + 2] for p in range(NPAIR)]
            mask_sb = constp.tile([128, 896], bf16, tag="mask", name="mask_sb")

            # gpsimd's DMA queue is a SLOW software DGE - nothing
            # startup-critical goes there. Granular transfers (128-256KB)
            # across the two fast queues keep the first matmuls fed early;
            # wq0 goes out first, split in halves.
            nc.scalar.dma_start(wq_sb[0][:, 0:512], wqk[0][:, 0:512])

            xc0 = []
            for t in range(DT_TILES):
                c0 = xp.tile([128, SQ], bf16, tag="x", name=f"x0_{t}")
                if t < 6:
                    nc.sync.dma_start(c0[:], xT[t][:, 0:SQ])
                xc0.append(c0)
            nc.scalar.dma_start(xc0[6][:], xT[6][:, 0:SQ])
            nc.scalar.dma_start(xc0[7][:], xT[7][:, 0:SQ])

            nc.scalar.dma_start(wq_sb[0][:, 512:1024], wqk[0][:, 512:1024])
            nc.scalar.dma_start(wk_sb[0][:, 0:512], wqk[0][:, 1024:1536])
            nc.scalar.dma_start(wk_sb[0][:, 512:1024], wqk[0][:, 1536:2048])
            nc.scalar.dma_start(wv_sb[:, 0:1024], wv[:, 0:1024])
            nc.scalar.dma_start(wv_sb[:, 1024:2048], wv[:, 1024:2048])
            nc.scalar.dma_start(wq_sb[1], wqk[1][:, 0:1024])
            nc.scalar.dma_start(wk_sb[1], wqk[1][:, 1024:2048])
            nc.scalar.dma_start(bqk_sb[:], bqk[:])

            # x for blocks 1-3: one wide DMA per d-tile; first tiles on the
            # fast sync queue
            nc.gpsimd.dma_start(mask_sb[:], maskd[:])
            xf = [qkstp.tile([128, S - SQ], bf16, tag=f"xf{t}", name=f"xf{t}")
                  for t in range(DT_TILES)]
            for t in range(DT_TILES):
                (nc.sync if t < 5 else nc.gpsimd).dma_start(
                    xf[t][:], xT[t][:, SQ:S])

            # PE warm-up: the HAM clock gate starts at 1.2 GHz and needs
            # ~3.4us of sustained activity to reach 2.4 GHz. Burn dummy
            # matmuls on a memset scratch while the first DMAs land so the
            # real projections start at full clock.
            # 15 dummies span the whole input-DMA window (~6.5us cold) so
            # the PE never idles (and re-cools) while transfers land
            warm = workp.tile([128, 512], bf16, tag="warm", bufs=1,
                              name="warm")
            nc.vector.memset(warm[:], 0.0)
            wps = ps.tile([128, 512], f32, tag="v", bufs=2, name="warmps")
            for i in range(15):
                nc.tensor.matmul(wps[:], warm[:, 0:128], warm[:],
                                 start=True, stop=True,
                                 skip_group_check=True)

            # ---- persistent activations ----
            qt_sb = [qkstp.tile([128, S], bf16, tag=f"qt{p}", name=f"qt{p}")
                     for p in range(NPAIR)]
            kt_sb = [qkstp.tile([128, S], bf16, tag=f"kt{p}", name=f"kt{p}")
                     for p in range(NPAIR)]
            zt_sb = [qkstp.tile([128, S], bf16, tag=f"zt{p}", name=f"zt{p}")
                     for p in range(NPAIR)]
            # V, augmented: head h at cols [128h, 128h+64), ones at
            # [128h+64, 128h+128) so the z-matmul replicates the softmax
            # denominator across PSUM partitions 64-127
            v_sb = [qkstp.tile([128, 512], bf16, tag=f"v{kt}", name=f"v{kt}")
                    for kt in range(ST128)]
            for kt in range(ST128):
                vo = v_sb[kt].rearrange("p (h c) -> p h c", c=128)
                ones_src = mask_sb[:, 512:768].rearrange(
                    "p (h c) -> p h c", c=64)
                nc.vector.tensor_copy(vo[:, :, 64:128], ones_src)

            # ---- phase 1 (per 512-wide s block): projections ----
            # returns a list of thunks so the caller can weave projection
            # chunks between attention steps (fills PE bubbles while exp runs)
            def proj_chunks(s4, xc_pre=None):
                sl = slice(s4 * SQ, (s4 + 1) * SQ)
                if xc_pre is not None:
                    xc = xc_pre
                else:
                    fsl = slice(s4 * SQ - SQ, (s4 + 1) * SQ - SQ)
                    xc = [xf[t][:, fsl] for t in range(DT_TILES)]

                def qk_chunk(p):
                    qk_ps = ps.tile([128, 2 * SQ], f32, tag="s", bufs=2,
                                    name=f"qkps{s4}{p}")
                    for t in range(DT_TILES):
                        nc.tensor.matmul(qk_ps[:, 0:SQ], wq_sb[p][:, 128 * t:128 * t + 128], xc[t][:],
                                         start=(t == 0), stop=(t == DT_TILES - 1))
                    for t in range(DT_TILES):
                        nc.tensor.matmul(qk_ps[:, SQ:2 * SQ], wk_sb[p][:, 128 * t:128 * t + 128], xc[t][:],
                                         start=(t == 0), stop=(t == DT_TILES - 1))
                    nc.vector.tensor_scalar_add(qt_sb[p][:, sl], qk_ps[:, 0:SQ],
                                                bq_sb[p][:, 0:1])
                    nc.vector.tensor_scalar_add(kt_sb[p][:, sl], qk_ps[:, SQ:2 * SQ],
                                                bk_sb[p][:, 0:1])

                def v_chunk(j):
                    kt = 4 * s4 + j
                    v_ps = ps.tile([128, 256], f32, tag="v", bufs=2, name=f"vps{kt}")
                    for t in range(DT_TILES):
                        nc.tensor.matmul(v_ps[:],
                                         xc[t][:, j * 128:(j + 1) * 128],
                                         wv_sb[:, 256 * t:256 * t + 256],
                                         start=(t == 0), stop=(t == DT_TILES - 1))
                    vo = v_sb[kt].rearrange("p (h c) -> p h c", c=128)
                    vs = v_ps.rearrange("p (h c) -> p h c", c=64)
                    nc.vector.tensor_copy(vo[:, :, 0:64], vs[:])

                return ([("qk", lambda p=p: qk_chunk(p)) for p in range(NPAIR)]
                        + [("v", lambda j=j: v_chunk(j)) for j in range(4)])

            # ---- phase 3 (emitted interleaved): output projection ----
            def oproj_group(st, half, otag, scalar_copy=False):
                ssl = slice(st * 128, (st + 1) * 128)
                dsl = slice(half * 512, (half + 1) * 512)
                o_ps = ps.tile([128, 512], f32, tag=otag,
                               bufs=2, name=f"ops{st}{half}")
                for p in range(NPAIR):
                    nc.tensor.matmul(o_ps[:], zt_sb[p][:, ssl],
                                     wo_sb[p][:, dsl],
                                     start=(p == 0), stop=(p == NPAIR - 1))
                o_sb = workp.tile([128, 512], f32, tag="osb",
                                  name=f"osb{st}{half}")
                if scalar_copy:
                    nc.scalar.activation(o_sb[:], o_ps[:], COPY)
                else:
                    nc.vector.tensor_copy(o_sb[:], o_ps[:])
                nc.sync.dma_start(out[st][:, dsl], o_sb[:])

            def emit_oproj(q4, otag="z", alt_copy=False):
                # alt_copy spreads the PSUM->SBUF copies over vector+scalar
                # (only safe when no exps remain on the scalar queue)
                i = 0
                for st in range(4 * q4, 4 * q4 + 4):
                    for half in range(2):
                        oproj_group(st, half, otag, alt_copy and i % 2 == 0)
                        i += 1

            def oproj_chunks(q4, otag):
                return [("v", lambda st=st, half=half: oproj_group(
                            st, half, otag))
                        for st in range(4 * q4, 4 * q4 + 4)
                        for half in range(2)]

            # ---- phase 2 (per 512-wide q block): attention ----
            def emit_attn(q4, weave):
                q0 = q4 * SQ
                qsl = slice(q0, q0 + SQ)
                nk = q4 * 4 + 4
                # qk chunks go at the head boundary (the block-end leftovers
                # of the previous weave already covered the previous norms);
                # then the previous block's output projection (tag z, whose
                # buffers those norms freed). The v chunks (own tag) weave
                # into the stream as PE filler, RESERVING two chunks for the
                # block end so the next block's head never waits on a bare
                # norm chain.
                while len(weave) > 2 and weave[0][0] != "v":
                    weave.pop(0)[1]()
                if 0 < q4 < NQ4 - 1:
                    emit_oproj(q4 - 1)
                elif q4 == NQ4 - 1:
                    # the last block's weave is the previous block's output
                    # projection; two chunks at the head cover the previous
                    # norms' PSUM recycling latency
                    for _ in range(2):
                        weave.pop(0)[1]()
                weave = [c for t, c in weave]
                steps = NPAIR * nk
                per = max(1, -(-steps // max(1, len(weave) - 2)))
                step = 0
                for p in range(NPAIR):
                    za = ps.tile([128, SQ], f32, tag="z", bufs=2, name=f"za{q4}{p}")
                    zb = ps.tile([128, SQ], f32, tag="z", bufs=2, name=f"zb{q4}{p}")
                    for kt in range(nk):
                        ksl = slice(kt * 128, (kt + 1) * 128)
                        d = kt * 128 - q0
                        first = (kt == 0)
                        s_ps = ps.tile([128, 2 * SQ], f32, tag="s",
                                       bufs=2, name=f"sps{q4}{p}{kt}")
                        e = workp.tile([128, 2 * SQ], bf16, tag="e",
                                       bufs=6, name=f"e{q4}{p}{kt}")
                        if d < 0:  # fully-allowed block: full-width scores
                            nc.tensor.matmul(s_ps[:, 0:SQ],
                                             kt_sb[p][0:64, ksl],
                                             qt_sb[p][0:64, qsl])
                            nc.tensor.matmul(s_ps[:, SQ:2 * SQ],
                                             kt_sb[p][64:128, ksl],
                                             qt_sb[p][64:128, qsl])
                            nc.scalar.activation(e[:], s_ps[:], EXP)
                        else:  # diagonal: only columns >= d survive the mask
                            nc.tensor.matmul(s_ps[:, d:SQ],
                                             kt_sb[p][0:64, ksl],
                                             qt_sb[p][0:64, q0 + d:q0 + SQ])
                            nc.tensor.matmul(s_ps[:, SQ + d:2 * SQ],
                                             kt_sb[p][64:128, ksl],
                                             qt_sb[p][64:128, q0 + d:q0 + SQ])
                            # one exp over both heads' trimmed ranges (3D AP)
                            e3 = e.rearrange("p (s q) -> p s q", q=SQ)
                            s3 = s_ps.rearrange("p (s q) -> p s q", q=SQ)
                            nc.scalar.activation(e3[:, :, d:SQ],
                                                 s3[:, :, d:SQ], EXP)
                        for sub, zps in ((0, za), (1, zb)):
                            h = 2 * p + sub
                            vap = v_sb[kt][:, 128 * h:128 * h + 128]
                            ebase = sub * SQ
                            if d < 0:  # fully-allowed block
                                nc.tensor.matmul(
                                    zps[:], vap, e[:, ebase:ebase + SQ],
                                    start=first, stop=False,
                                    skip_group_check=True)
                            else:
                                # columns [0, d) fully masked: skip.
                                # columns [d, d+128): mixed - mask-multiply.
                                em = workp.tile([128, 128], bf16, tag="em",
                                                bufs=6, name=f"em{q4}{p}{kt}{sub}")
                                nc.vector.tensor_mul(
                                    em[:], e[:, ebase + d:ebase + d + 128],
                                    mask_sb[:, 384:512])
                                nc.tensor.matmul(
                                    zps[:, d:d + 128], vap, em[:],
                                    start=first, stop=True,
                                    skip_group_check=True)
                                # columns [d+128, 512): fully allowed.
                                if d + 128 < SQ:
                                    nc.tensor.matmul(
                                        zps[:, d + 128:SQ], vap,
                                        e[:, ebase + d + 128:ebase + SQ],
                                        start=first, stop=False,
                                        skip_group_check=True)
                        step += 1
                        if len(weave) > 2 and step % per == 0:
                            weave.pop(0)()
                    if q4 == NQ4 - 1 and p == NPAIR - 1:
                        # the very last normalization gates the final output
                        # projection: chunk it per 128-col s-tile with the
                        # copies on the (now exp-free) scalar engine, so the
                        # oproj that follows pipelines with it per s-tile
                        for j in range(4):
                            csl = slice(j * 128, (j + 1) * 128)
                            gsl = slice(q0 + j * 128, q0 + (j + 1) * 128)
                            for sub, zps in ((0, za), (1, zb)):
                                den = workp.tile([64, 128], f32, tag="den",
                                                 bufs=4, name=f"dc{j}{sub}")
                                nc.scalar.activation(den[:],
                                                     zps[64:128, csl], COPY)
                                rb = workp.tile([64, 128], f32, tag="rb",
                                                bufs=4, name=f"rc{j}{sub}")
                                nc.vector.reciprocal_approx_fast(rb[:], den[:])
                                nc.vector.tensor_mul(
                                    zt_sb[p][64 * sub:64 * sub + 64, gsl],
                                    zps[0:64, csl], rb[:])
                    else:
                        for sub, zps in ((0, za), (1, zb)):
                            # rows 64-127 hold the denominator replicated by
                            # the ones half of V_aug: DVE copy to SBUF
                            # (custom-DVE recip can't read PSUM; keep scalar
                            # free for exp), DVE recip, full-width multiply
                            den = workp.tile([64, SQ], f32, tag="den", bufs=4,
                                             name=f"den{q4}{p}{sub}")
                            nc.vector.tensor_copy(den[:], zps[64:128, :])
                            rb = workp.tile([64, SQ], f32, tag="rb", bufs=4,
                                            name=f"rb{q4}{p}{sub}")
                            nc.vector.reciprocal_approx_fast(rb[:], den[:])
                            nc.vector.tensor_mul(
                                zt_sb[p][64 * sub:64 * sub + 64, qsl],
                                zps[0:64, :], rb[:])
                    # cover the pair-boundary PSUM recycling stall with a
                    # woven chunk (pure PE work with its own PSUM tag)
                    if p == 0 and len(weave) > 2:
                        weave.pop(0)()
                # block-end leftovers: PE filler between this block's last z
                # matmuls / norms and whatever the caller emits next
                while weave:
                    weave.pop(0)()

            # fully interleave: projections for x-block s4+1 are woven into
            # the attention stream of q-block s4 (whose deps only reach s4),
            # so the PSUM tag rotation flows without a phase barrier and
            # matmul-dense projection chunks fill exp-bound PE bubbles.
            for _, c in proj_chunks(0, xc_pre=xc0):
                c()
            # W_O not needed until the first o-proj
            nc.gpsimd.dma_start(wo_sb_[:], wo[:])
            for q4 in range(NQ4):
                if q4 + 1 < NQ4:
                    nxt = proj_chunks(q4 + 1)
                else:
                    # the last block has no projections left to weave; use
                    # the previous block's output projection (the idle "v"
                    # PSUM rotation) as its PE filler instead
                    nxt = oproj_chunks(q4 - 1, otag="v")
                emit_attn(q4, nxt)
            emit_oproj(NQ4 - 1, otag="v", alt_copy=True)

    nc.compile()
    return nc


def _get_built():
    global _BUILT
    if _BUILT is None:
        _BUILT = _build()
    return _BUILT


def _host_prep(x, W_Q, W_K, W_V, W_O, b_Q, b_K):
    """Build the 8 per-core input maps."""
    import ml_dtypes
    bf16 = ml_dtypes.bfloat16
    scale = np.float32(1.0 / np.sqrt(Dh))
    mask = (np.arange(896)[None, :] >= (np.arange(128)[:, None] + 384)
            ).astype(bf16)
    in_maps = []
    for c in range(NCORES):
        b = c // CORES_PER_BATCH
        g = c % CORES_PER_BATCH
        hs = slice(HPC * g, HPC * g + HPC)
        xT_b = np.ascontiguousarray(x[b].T).reshape(
            DT_TILES, 128, S).astype(bf16)
        def pack_de(w):
            # [4 heads, D, Dh] -> pair-stacked [2, D, 128] -> [2, 128, 8*128]
            a = w.reshape(NPAIR, 2, D, Dh).transpose(0, 2, 1, 3).reshape(
                NPAIR, DT_TILES, 128, 128)
            return np.ascontiguousarray(a.transpose(0, 2, 1, 3)).reshape(
                NPAIR, 128, 1024).astype(bf16)

        wq_c = pack_de(W_Q[hs] * scale)
        wk_c = pack_de(W_K[hs])
        wqk_c = np.ascontiguousarray(
            np.concatenate([wq_c, wk_c], axis=2))
        wv_c = np.ascontiguousarray(
            W_V[hs].transpose(1, 0, 2).reshape(DT_TILES, 128, HPC * Dh)
            .transpose(1, 0, 2)).reshape(128, 2048).astype(bf16)
        wo_p = np.ascontiguousarray(W_O[hs]).reshape(NPAIR, 128, 1024)
        wo_c = np.ascontiguousarray(
            np.concatenate([wo_p[0], wo_p[1]], axis=1)).astype(bf16)
        bq_c = np.ascontiguousarray(b_Q[hs] * scale).reshape(NPAIR, 128)
        bk_c = np.ascontiguousarray(b_K[hs]).reshape(NPAIR, 128)
        bqk_c = np.ascontiguousarray(np.stack(
            [bq_c[0], bk_c[0], bq_c[1], bk_c[1]], axis=1)).astype(np.float32)
        in_maps.append({
            "xT": xT_b, "wqk": wqk_c, "wv": wv_c, "wo": wo_c,
            "bqk": bqk_c, "maskd": mask,
        })
    return in_maps


def kernel(normalized_resid_pre, W_Q, W_K, W_V, W_O, b_Q, b_K, b_V, b_O,
           _want_profile=False):
    from concourse.bass_utils import run_bass_kernel_spmd

    x = np.asarray(normalized_resid_pre, np.float32)
    W_Q = np.asarray(W_Q, np.float32)
    W_K = np.asarray(W_K, np.float32)
    W_V = np.asarray(W_V, np.float32)
    W_O = np.asarray(W_O, np.float32)
    b_Q = np.asarray(b_Q, np.float32)
    b_K = np.asarray(b_K, np.float32)
    b_V = np.asarray(b_V, np.float32)
    b_O = np.asarray(b_O, np.float32)

    in_maps = _host_prep(x, W_Q, W_K, W_V, W_O, b_Q, b_K)
    nc = _get_built()
    kw = {}
    if _want_profile:
        kw = dict(trace=True)
    res = run_bass_kernel_spmd(nc, in_maps, list(range(NCORES)), **kw)

    # host-side unshard: sum the head-group partials per batch + bias terms
    b_eff = b_O + np.einsum("he,hed->d", b_V, W_O).astype(np.float32)
    attn_out = np.zeros((B, S, D), np.float32)
    for c in range(NCORES):
        b = c // CORES_PER_BATCH
        attn_out[b] += res.results[c]["out"].reshape(S, D)
    attn_out += b_eff[None, None, :]
    if _want_profile:
        return attn_out, res
    return attn_out
